# revision 1
# baseline (speedup 1.0000x reference)
"""AttentionReadout kernel for 8 trn2 NeuronCores (v2).

Problem: gate-MLP attention readout over 500k nodes, D=256, G=1024 graphs.
    h = tanh(x @ W1 + b1); s = h @ W2 + b2
    attn = segment_softmax(s, batch); out[g] = sum_{n in g} attn[n] * x[n]
(b2 cancels inside the per-graph softmax, so it is dropped.)

Design notes (bf16 data / f32 PSUM accumulation throughout):
  - Node-sharding: 8 equal slabs of 62500 nodes padded to 62976 (123 groups
    of 512) -> one SPMD program for all cores.  Softmax normalization and
    graph gathering happen on the host from unnormalized partial sums and
    the per-node e values, so graphs split across cores/chunks are exact.
  - x^T for the gate MLP comes two ways to balance PE against the DMA
    engines: route-A groups PE-transpose the bf16 x tiles (+DVE copy to
    SBUF); route-B groups (45/123) DMA a host-pretransposed fp8 hi|lo pair
    (same bytes as bf16) and run layer 1 as 3 fp8 DoubleRow matmuls
    (hi*hi + hi*lo + lo*hi of 16x-scaled operands, undone by the tanh
    `scale`), which is as accurate as bf16 and half the PE cycles.
  - One tanh per group: d_out is permuted by argsort(b1) and paired so both
    output halves share a single per-partition bias.  Per-chunk 1-column
    score matmuls land in a persistent PSUM bank; exp runs per 3-group
    window into a persistent e slab that is also an output (the host builds
    softmax denominators from it).
  - Pooling: per-chunk columns.  Each 128-node chunk spans <=kcols (=2)
    graphs; a [128, kcols] masked-e "onehot" (built 8 chunks at a time in
    two broadcast DVE ops) is the *moving* operand against the x chunk as
    *stationary*, so pooling costs ~kcols PE cycles per chunk.  Partials
    accumulate in a 1-bank PSUM window per 8 chunks, are flushed to an SBUF
    slab, and stream to DRAM via the otherwise-idle GPSIMD DGE.
  - Software pipelining: input DMAs issue 6 groups ahead (hiding the 900ns
    DMA semaphore latency), transposes+copies run one group ahead of the u
    matmuls, scores lag tanh by one group, and each window's onehot/pooling
    burst trails its exp by ~4 steps, so no engine blocks in-order on
    another.  Mid-run output flushes ride the otherwise-idle GPSIMD DGE so
    their waits never stall SP's input-DMA stream; the final flush and the
    e slab go via SP at the tail.
"""

import sys

sys.path.insert(0, "/opt/trn_rl_repo")

from contextlib import ExitStack

import numpy as np
import ml_dtypes

import concourse.bass as bass
import concourse.tile as tile
from concourse import mybir
from concourse.bass_utils import run_bass_kernel_spmd

N_NODES = 500_000
D = 256
G = 1024
N_CORES = 8
CHUNK = 128
GROUP = 4  # chunks per group (512 nodes)
NPG = CHUNK * GROUP
QUAD = 3  # groups per pool window/burst (12 chunks)
DCX = D  # x row (denominators come from the e output on host)
NPC = N_NODES // N_CORES  # 62500 real nodes per core
N_GROUPS = -(-NPC // NPG)  # 123
NPAD = N_GROUPS * NPG  # 62976
N_CHUNKS = NPAD // CHUNK  # 492
BF16 = ml_dtypes.bfloat16


def _split_waits(nc, max_waits=1):
    """Hoist extra semaphore waits onto preceding same-engine NOPs.

    The walrus build in this container rejects instructions carrying more
    than one embedded sync wait; engines execute their stream in order, so a
    wait on a preceding NOP is equivalent.
    """
    n = 0
    for fn in nc.m.functions:
        for blk in fn.blocks:
            newlist = []
            for ins in blk.instructions:
                si = ins.sync_info
                if si is not None and len(si.on_wait) > max_waits:
                    waits = list(si.on_wait)
                    keep, extra = waits[:max_waits], waits[max_waits:]
                    for w in extra:
                        n += 1
                        nop = mybir.InstNoOp(
                            name=f"waitsplit-{n}-{ins.name}", ins=[], outs=[]
                        )
                        nop.engine = ins.engine
                        nop.sync_info = mybir.SyncInfo(on_wait=[w], on_update=[])
                        nc.register_instruction(nop, overwrite=True)
                        newlist.append(nop)
                    ins.sync_info = mybir.SyncInfo(
                        on_wait=keep, on_update=list(si.on_update)
                    )
                newlist.append(ins)
            blk.instructions[:] = newlist
    return n


N_B = 45  # route-B groups (of 123): balances PE against the DMA engines


def _route_b(t):
    """Groups whose x^T tile comes as a pre-transposed fp8 hi|lo pair from
    DRAM and run the first MLP layer as 3 fp8 DoubleRow matmuls; the rest
    transpose x on the PE and run bf16 matmuls."""
    return t == 0 or ((t + 1) * N_B) % N_GROUPS < N_B


def build_nc(kcols, split=True):
    f32 = mybir.dt.float32
    bf16 = mybir.dt.bfloat16
    n_b = sum(1 for t in range(N_GROUPS) if _route_b(t))
    ncols = kcols * N_CHUNKS  # pooling output columns
    wincols = kcols * QUAD * GROUP  # pooling columns per window

    fp8 = mybir.dt.float8e4
    nc = bass.Bass()
    x_d = nc.declare_dram_parameter("x", [CHUNK, N_CHUNKS, DCX], bf16, isOutput=False)
    # route-B x^T: [partition, k-half, slot, hi|lo, nodes] fp8 (x scaled by 16)
    xt_d = nc.declare_dram_parameter("xt", [CHUNK, 2, n_b, 2, NPG], fp8, isOutput=False)
    # route-B W1: per (hi|lo, m-half) a [128, 2, 128] DoubleRow block (W1 scaled by 16)
    cb8_d = nc.declare_dram_parameter("cb8", [CHUNK, 4, 2, CHUNK], fp8, isOutput=False)
    # constants packed into one bf16 and one f32 DMA:
    # cb16 = [iota | ident | w1 | w2 | bid], cf32 = [b1]
    nb16 = kcols + CHUNK + 4 * CHUNK + 2 + N_CHUNKS
    cb_d = nc.declare_dram_parameter("cb16", [CHUNK, nb16], bf16, isOutput=False)
    cf_d = nc.declare_dram_parameter("cf32", [CHUNK, 1], f32, isOutput=False)
    out_d = nc.declare_dram_parameter("out", [CHUNK, 2, ncols], bf16, isOutput=True)
    e_d = nc.declare_dram_parameter("e", [CHUNK, N_CHUNKS], bf16, isOutput=True)

    with tile.TileContext(nc) as tc, ExitStack() as ctx:
        const = ctx.enter_context(tc.tile_pool(name="const", bufs=1))
        xwp = ctx.enter_context(tc.tile_pool(name="xw", bufs=13))
        xtp = ctx.enter_context(tc.tile_pool(name="xts", bufs=8))
        hp = ctx.enter_context(tc.tile_pool(name="h", bufs=7))
        ohp = ctx.enter_context(tc.tile_pool(name="oh", bufs=3))
        sab = ctx.enter_context(tc.tile_pool(name="sab", bufs=1))
        ps_u = ctx.enter_context(tc.tile_pool(name="ps_u", bufs=2, space="PSUM"))
        ps_xt = ctx.enter_context(tc.tile_pool(name="ps_xt", bufs=2, space="PSUM"))
        ps_s = ctx.enter_context(tc.tile_pool(name="ps_s", bufs=1, space="PSUM"))
        ps_w = ctx.enter_context(tc.tile_pool(name="ps_w", bufs=1, space="PSUM"))

        # Resident constants (packed views; DMAs issued after the first x
        # prefetches below so the compute pipeline fills immediately)
        cb = const.tile([CHUNK, nb16], bf16, tag="cb16", name="cb16")
        cb8 = const.tile([CHUNK, 4, 2, CHUNK], fp8, tag="cb8", name="cb8")
        cf = const.tile([CHUNK, 1], f32, tag="cf32", name="cf32")
        iota_t = cb[:, 0:kcols]
        ident_t = cb[:, kcols : kcols + CHUNK]
        w1_base = kcols + CHUNK

        def w1_blk(s):
            return cb[:, w1_base + s * CHUNK : w1_base + (s + 1) * CHUNK]

        w2_t = cb[:, w1_base + 4 * CHUNK : w1_base + 4 * CHUNK + 2]
        bid_t = cb[:, w1_base + 4 * CHUNK + 2 : w1_base + 4 * CHUNK + 2 + N_CHUNKS]
        b1_t = cf[:, 0:1]

        # Persistent score bank, e slab, and output slab
        s_all = ps_s.tile([CHUNK, N_CHUNKS], f32, tag="s_all", name="s_all")
        e_slab = sab.tile([CHUNK, N_CHUNKS], bf16, tag="e_slab", name="e_slab")
        psab = sab.tile([CHUNK, 2, ncols], bf16, tag="psab", name="psab")

        xw_tiles = {}  # pair index -> tile
        hT_tiles = {}

        def pool_burst(q, groups):
            """onehot + pooling matmuls + window flush for a quad."""
            nchk = GROUP * len(groups)
            c0 = groups[0] * GROUP
            pw = ps_w.tile([CHUNK, 2, wincols], f32, tag="pw", name="pw")
            nc.vector.memset(pw[:], 0.0)
            # masked-e "onehot" for the whole quad in two DVE ops:
            # oh[p, i, j] = (bid[p, c0+i] == j) * e[p, i]
            shp = [CHUNK, nchk, kcols]
            oh = ohp.tile(shp, bf16, tag="oh", name="oh")
            nc.vector.tensor_tensor(
                oh[:],
                bid_t[:, c0 : c0 + nchk].unsqueeze(2).broadcast_to(shp),
                iota_t.unsqueeze(1).broadcast_to(shp),
                mybir.AluOpType.is_equal,
            )
            nc.vector.tensor_tensor(
                oh[:],
                oh[:],
                e_slab[:, c0 : c0 + nchk].unsqueeze(2).broadcast_to(shp),
                mybir.AluOpType.mult,
            )
            for i in range(nchk):
                cc = c0 + i
                pr, off = cc // (2 * GROUP), cc % (2 * GROUP)
                xw = xw_tiles[pr]
                col = kcols * i
                for half in range(2):
                    nc.tensor.matmul(
                        pw[:, half, col : col + kcols],
                        xw[:, off, half * CHUNK : (half + 1) * CHUNK],
                        oh[:, i, :],
                        start=False,
                        stop=False,
                        skip_group_check=True,
                    )
            # flush window to the SBUF slab
            nc.vector.tensor_copy(
                psab[:, :, kcols * c0 : kcols * (c0 + nchk)], pw[:, :, 0 : kcols * nchk]
            )

        xts_tiles = {}

        def do_dma(t, ng=2):
            """Input DMAs for group t, issued PF groups ahead of compute to
            hide the transfer + 900ns DMA semaphore latency."""
            if t % 2 == 0:
                pr = t // 2
                ng = min(ng, N_GROUPS - t)  # groups in this x DMA
                xw = xwp.tile(
                    [CHUNK, 2 * GROUP, DCX], bf16, tag="xw", name=f"xw{pr}"
                )
                nc.sync.dma_start(
                    xw[:, 0 : ng * GROUP, :],
                    x_d[:, t * GROUP : (t + ng) * GROUP, :],
                )
                xw_tiles[pr] = xw
            if _route_b(t):
                slot = sum(1 for u in range(t) if _route_b(u))
                xts = xtp.tile([CHUNK, 2, 2, NPG], fp8, tag="xt8", name="xt8")
                nc.sync.dma_start(xts[:], xt_d[:, :, slot, :, :])
                xts_tiles[t] = xts

        def do_xts(t):
            """Transposes + PSUM->SBUF copy for route-A groups, one group
            ahead of the u matmuls that consume the result."""
            if _route_b(t):
                return
            xw = xw_tiles[t // 2]
            off0 = (t % 2) * GROUP
            xt_ps = ps_xt.tile([CHUNK, 2, NPG], bf16, tag="xt_ps", name="xt_ps")
            for k in range(2):
                for j in range(GROUP):
                    nc.tensor.transpose(
                        xt_ps[:, k, j * CHUNK : (j + 1) * CHUNK],
                        xw[:, off0 + j, k * CHUNK : (k + 1) * CHUNK],
                        ident_t[:],
                    )
            xts = xtp.tile([CHUNK, 2, NPG], bf16, tag="xts", name="xts")
            nc.vector.tensor_copy(xts[:], xt_ps[:])
            xts_tiles[t] = xts

        def do_group(t):
            xts = xts_tiles.pop(t)
            u_ps = ps_u.tile([CHUNK, 2, NPG], f32, tag="u", name="u")
            if _route_b(t):
                # u*256 = sum of 3 fp8 DoubleRow products (hi*hi, hi*lo, lo*hi)
                for m in range(2):
                    for i, (wi, xi) in enumerate(((0, 0), (1, 0), (0, 1))):
                        nc.tensor.matmul(
                            u_ps[:, m, :],
                            cb8[:, 2 * wi + m, :, :],
                            xts[:, :, xi, :],
                            start=(i == 0),
                            stop=(i == 2),
                            perf_mode=mybir.MatmulPerfMode.DoubleRow,
                        )
                scale = 1.0 / 256.0
            else:
                for k in range(2):
                    for m in range(2):
                        nc.tensor.matmul(
                            u_ps[:, m, :],
                            w1_blk(2 * k + m),
                            xts[:, k, :],
                            start=(k == 0),
                            stop=(k == 1),
                        )
                scale = 1.0
            hT = hp.tile([CHUNK, 2, NPG], bf16, tag="hT", name="hT")
            nc.scalar.activation(
                hT[:],
                u_ps[:],
                mybir.ActivationFunctionType.Tanh,
                bias=b1_t[:, 0:1],
                scale=scale,
            )
            hT_tiles[t] = hT

        def do_scores(t):
            hT = hT_tiles.pop(t)
            for j in range(GROUP):
                for m in range(2):
                    nc.tensor.matmul(
                        s_all[:, t * GROUP + j : t * GROUP + j + 1],
                        hT[:, m, j * CHUNK : (j + 1) * CHUNK],
                        w2_t[:, m : m + 1],
                        start=False,
                        stop=False,
                        skip_group_check=True,
                    )

        def do_exp(c0, c1):
            nc.scalar.activation(
                e_slab[:, c0:c1],
                s_all[:, c0:c1],
                mybir.ActivationFunctionType.Exp,
            )

        # Software pipeline: scores lag one group behind tanh, and each
        # quad's exp/onehot/pooling burst lags two groups behind its last
        # score, so PE never blocks in-order on the Act engine.
        quads = []
        for q0 in range(0, N_GROUPS, QUAD):
            quads.append((q0 // QUAD, list(range(q0, min(q0 + QUAD, N_GROUPS)))))
        n_quads = len(quads)
        PF = 6  # groups of DMA lookahead
        EXPG = 2 * QUAD  # groups per exp instruction (2 quads)

        # schedule: exp for a quad-pair fires one step after the pair's last
        # scores; each of its bursts one and two steps later.
        burst_at = {}
        exp_at = {}
        for q in range(n_quads):
            last_group = quads[q][1][-1]
            te = last_group + 2  # step emitting this window's exp
            exp_at[te] = (quads[q][1][0] * GROUP, (last_group + 1) * GROUP)
            burst_at.setdefault(te + 4, []).append(q)
        t_end = max(burst_at) + 1
        last_exp_t = max(exp_at)

        flushed = [0]  # pooling columns already flushed / dmaed out

        def out_flush(upto_col):
            # issued on the otherwise-idle GPSIMD engine so the wait for
            # pooling columns never blocks SP's input-DMA stream
            a = flushed[0]
            if upto_col > a:
                nc.gpsimd.dma_start(out_d[:, :, a:upto_col], psab[:, :, a:upto_col])
                flushed[0] = upto_col

        # group-0 inputs and their weights first, then the other constants,
        # then the rest of the prefetch window.
        nc.sync.dma_start(cb8[:], cb8_d[:])
        do_dma(0)
        nc.sync.dma_start(cb[:], cb_d[:])
        nc.sync.dma_start(cf[:], cf_d[:])
        nc.vector.memset(s_all[:], 0.0)
        for td in range(1, min(PF, N_GROUPS)):
            do_dma(td)

        for t in range(0, t_end + 1):
            if t + PF < N_GROUPS:
                do_dma(t + PF)
            if t == 0:
                do_xts(0)
            if t + 1 < N_GROUPS:
                do_xts(t + 1)
            if t < N_GROUPS:
                do_group(t)
            if 0 <= t - 1 < N_GROUPS:
                do_scores(t - 1)
            if t in exp_at:
                do_exp(*exp_at[t])
            for q in burst_at.get(t, ()):
                pool_burst(*quads[q])
                if q % 8 == 7 or False:
                    out_flush(min(ncols, kcols * QUAD * GROUP * (q + 1)))
        a = flushed[0]
        nc.sync.dma_start(out_d[:, :, a:ncols], psab[:, :, a:ncols])
        nc.sync.dma_start(e_d[:], e_slab[:])

    if split:
        _split_waits(nc)
    return nc


def prepare_inputs(x, batch, W1, b1, W2, b2):
    """Host-side sharding and layout preparation."""
    x = np.asarray(x, dtype=np.float32)
    batch = np.asarray(batch).astype(np.int64)
    W1 = np.asarray(W1, dtype=np.float32)
    b1 = np.asarray(b1, dtype=np.float32).reshape(D)
    W2 = np.asarray(W2, dtype=np.float32).reshape(D)

    # Pair d_out dims by sorted b1 so one per-partition bias serves both
    # tanh output halves (pairing error ~1e-4, far below bf16 noise).
    perm = np.argsort(b1, kind="stable")
    colmap = np.empty(D, np.int64)
    for m in range(2):
        colmap[m * CHUNK : (m + 1) * CHUNK] = perm[m::2]
    W1P = W1[:, colmap].astype(BF16)
    b1s = b1[perm]
    b1bar = ((b1s[0::2] + b1s[1::2]) / 2).astype(np.float32).reshape(CHUNK, 1)
    w1t = np.empty((CHUNK, 4, CHUNK), BF16)
    for k in range(2):
        for m in range(2):
            w1t[:, 2 * k + m, :] = W1P[
                k * CHUNK : (k + 1) * CHUNK, m * CHUNK : (m + 1) * CHUNK
            ]
    # fp8 hi|lo split of 16*W1P for the DoubleRow path
    FP8 = ml_dtypes.float8_e4m3fn
    ws = W1P.astype(np.float32) * 16.0
    w8 = [ws.astype(FP8)]
    w8.append((ws - w8[0].astype(np.float32)).astype(FP8))
    cb8 = np.empty((CHUNK, 4, 2, CHUNK), FP8)
    for wi in range(2):
        for m in range(2):
            for kh in range(2):
                cb8[:, 2 * wi + m, kh, :] = w8[wi][
                    kh * CHUNK : (kh + 1) * CHUNK, m * CHUNK : (m + 1) * CHUNK
                ]
    w2t = np.ascontiguousarray(
        W2[colmap].astype(BF16).reshape(2, CHUNK).T
    )  # w2t[p, m] = W2[colmap[m*128+p]]

    ident = np.eye(CHUNK, dtype=BF16)
    n_b = sum(1 for t in range(N_GROUPS) if _route_b(t))
    b_slots = [t for t in range(N_GROUPS) if _route_b(t)]

    in_maps = []
    gmaps = []
    kcols_all = 1
    cores = []
    for c in range(N_CORES):
        r0 = c * NPC
        r1 = min(N_NODES, r0 + NPC)
        n = r1 - r0
        arr = np.zeros((NPAD, DCX), dtype=BF16)
        arr[:n, :D] = x[r0:r1].astype(BF16)
        x_nat = np.ascontiguousarray(
            arr.reshape(N_CHUNKS, CHUNK, DCX).transpose(1, 0, 2)
        )
        # fp8 hi|lo split of 16*x (from the same bf16 values the pool uses)
        xsc = arr.astype(np.float32) * 16.0
        xhi = xsc.astype(FP8)
        xlo = (xsc - xhi.astype(np.float32)).astype(FP8)
        x8 = np.stack([xhi, xlo], axis=1)  # [NPAD, 2, D]
        # -> [128, kh, group, hi|lo, node]
        xt = np.ascontiguousarray(
            x8.reshape(N_GROUPS, NPG, 2, 2, CHUNK).transpose(4, 3, 0, 2, 1)[
                :, :, b_slots, :, :
            ]
        )

        b = batch[r0:r1]
        b_pad = np.full(NPAD, -1, np.int64)
        b_pad[:n] = b
        gf = b_pad[:: CHUNK].copy()  # first graph id per chunk (-1 if pad)
        cidx = np.arange(NPAD) // CHUNK
        gf_c = np.where(gf >= 0, gf, 0)
        bid = np.where(b_pad >= 0, b_pad - gf_c[cidx], -1).astype(np.int64)
        kc = int(bid.max()) + 1
        kcols_all = max(kcols_all, kc)
        cores.append((x_nat, xt, bid.astype(np.float32).reshape(N_CHUNKS, CHUNK).T, gf))

    kcols = max(2, kcols_all)
    iota = np.broadcast_to(np.arange(kcols, dtype=BF16), (CHUNK, kcols))
    cb_common = np.concatenate(
        [iota, ident, w1t.reshape(CHUNK, 4 * CHUNK), w2t], axis=1
    ).astype(BF16)
    for c in range(N_CORES):
        x_nat, xt, bid2d, gf = cores[c]
        cb16 = np.concatenate([cb_common, bid2d.astype(BF16)], axis=1)
        in_maps.append(
            {
                "x": x_nat,
                "xt": xt,
                "cb16": np.ascontiguousarray(cb16),
                "cb8": cb8,
                "cf32": np.ascontiguousarray(b1bar),
            }
        )
        # host mapping: column kcols*c + j -> graph gf[c] + j
        gmap = np.full((N_CHUNKS, kcols), -1, np.int64)
        for j in range(kcols):
            gj = gf + j
            gmap[:, j] = np.where((gf >= 0) & (gj < G), gj, -1)
        gmaps.append(gmap)
    return in_maps, gmaps, kcols


def postprocess(results, gmaps, batch, kcols):
    batch = np.asarray(batch).astype(np.int64)
    pool = np.zeros((G, D), np.float64)
    den = np.zeros(G, np.float64)
    for c in range(N_CORES):
        res = np.asarray(results[c]["out"], dtype=np.float64)  # [128, 2, ncols]
        gm = gmaps[c].ravel()
        valid = gm >= 0
        idx = gm[valid]
        np.add.at(pool[:, :CHUNK], idx, res[:, 0, valid].T)
        np.add.at(pool[:, CHUNK:], idx, res[:, 1, valid].T)
        # denominators from the per-node e values (same bf16 values the
        # device pooled with)
        e_arr = np.asarray(results[c]["e"], dtype=np.float64)  # [128, n_chunks]
        e_node = e_arr.T.ravel()  # node order within this core
        r0 = c * NPC
        r1 = min(N_NODES, r0 + NPC)
        np.add.at(den, batch[r0:r1], e_node[: r1 - r0])
    out = np.where(den[:, None] > 0, pool / np.maximum(den, 1e-300)[:, None], 0.0)
    return out.astype(np.float32)


def kernel(x, batch, num_graphs, W1, b1, W2, b2):
    assert int(num_graphs) == G
    in_maps, gmaps, kcols = prepare_inputs(x, batch, W1, b1, W2, b2)
    nc = build_nc(kcols)
    res = run_bass_kernel_spmd(nc, in_maps, list(range(N_CORES)))
    return postprocess(res.results, gmaps, batch, kcols)



# revision 51
# speedup vs baseline: 1.0551x; 1.0551x over previous
"""AttentionReadout kernel for 8 trn2 NeuronCores (v3).

Problem: gate-MLP attention readout over 500k nodes, D=256, G=1024 graphs.
    h = tanh(x @ W1 + b1); s = h @ W2 + b2
    attn = segment_softmax(s, batch); out[g] = sum_{n in g} attn[n] * x[n]
(b2 cancels inside the per-graph softmax, so it is dropped.)

v3 design notes (bf16 data / f32 PSUM accumulation):
  - Node-sharding: 8 equal slabs of 62500 nodes padded to 62976 (123 groups
    of 512) -> one SPMD program for all cores.  Softmax normalization and
    graph gathering happen on the host from unnormalized partial sums and
    the per-node e values, so graphs split across cores/chunks are exact.
  - Three per-group routes balance PE against the DMA engines:
      A : x^T by PE-transposing the bf16 x tiles (+DVE copy to SBUF);
          layer 1 as 4 bf16 matmuls (W1 host-scaled by 256 so one tanh
          scale serves all routes).
      B1: DMA a host-pretransposed fp8 hi|lo pair (same bytes as bf16);
          layer 1 as 3 fp8 DoubleRow matmuls of 16x-scaled operands --
          bf16-accurate at half the PE cycles of route A.
      B2: DMA only the fp8 hi part (half the bytes of B1); layer 1 as 2
          DoubleRow matmuls.  The ~2.6% per-element x quantization noise
          only perturbs gate scores (softmax weights), contributing ~1%
          relative output error; pooling still uses exact bf16 x.
  - The Act engine is the critical resource (123 per-group tanh ops run
    back to back); u tiles are triple-buffered in PSUM (6 banks) so, with
    the tile-granular dependency tracking, each group's layer-1 matmuls
    only wait on the tanh three groups back.  Route-A transposes stage the
    k0 half in the group's own u tile (bitcast bf16 view, overwritten by
    the u matmuls right after the copy drains it) and the k1 half in a
    dedicated staging bank; the two PSUM->SBUF half copies pipeline with
    the transposes so the transpose->copy->matmul chain fits inside the
    three-tanh budget.  d_out is permuted by argsort(b1) and paired so
    both tanh output halves share a single per-partition bias.
  - Scores: per-chunk 1-column matmuls (start/stop per column, no memset)
    into a 48-column PSUM window sharing the last bank with the pooling
    window; exp runs per ~4 quads into a persistent e slab that is also an
    output (the host builds softmax denominators from it).  Scores lag
    their tanh by 2 steps (3 at block starts) so neither the hT wait nor
    the window WAR on the previous exp ever stalls PE's in-order stream
    ahead of the next group's u matmuls.
  - Pooling: per-chunk columns.  Each 128-node chunk spans <=kcols (=2)
    graphs; a [128, kcols] masked-e "onehot" (built 12 chunks at a time in
    two broadcast DVE ops) is the *moving* operand against the x chunk as
    *stationary*, so pooling costs ~kcols PE cycles per chunk.  Partials
    land in the shared PSUM window (start=stop=True, each column written
    once), are flushed to an SBUF slab, and stream to DRAM via the
    otherwise-idle GPSIMD DGE.  Bursts run ~5 steps behind their exp so
    their x tiles and the window-bank WAR with the next exp never gate the
    tanh stream; the drain keeps them tight instead.
  - DMAs are issued in first-use order (route-A x pairs ahead of their
    transposes, pooling-only x pairs much later); the head runs on B2
    groups whose tiny xt DMAs fill the pipeline fastest.
"""

import sys

sys.path.insert(0, "/opt/trn_rl_repo")

from contextlib import ExitStack

import numpy as np
import ml_dtypes

import concourse.bass as bass
import concourse.tile as tile
from concourse import mybir
from concourse.bass_utils import run_bass_kernel_spmd

N_NODES = 500_000
D = 256
G = 1024
N_CORES = 8
CHUNK = 128
GROUP = 4  # chunks per group (512 nodes)
NPG = CHUNK * GROUP
QUAD = 3  # groups per pool window/burst (12 chunks)
DCX = D  # x row (denominators come from the e output on host)
NPC = N_NODES // N_CORES  # 62500 real nodes per core
N_GROUPS = -(-NPC // NPG)  # 123
NPAD = N_GROUPS * NPG  # 62976
N_CHUNKS = NPAD // CHUNK  # 492
BF16 = ml_dtypes.bfloat16
SWIN = 192  # rolling score-window columns (8 sub-rows of 24)


def _split_waits(nc, max_waits=1):
    """Hoist extra semaphore waits onto preceding same-engine NOPs.

    The walrus build in this container rejects instructions carrying more
    than one embedded sync wait; engines execute their stream in order, so a
    wait on a preceding NOP is equivalent.
    """
    n = 0
    for fn in nc.m.functions:
        for blk in fn.blocks:
            newlist = []
            for ins in blk.instructions:
                si = ins.sync_info
                if si is not None and len(si.on_wait) > max_waits:
                    waits = list(si.on_wait)
                    keep, extra = waits[:max_waits], waits[max_waits:]
                    for w in extra:
                        n += 1
                        nop = mybir.InstNoOp(
                            name=f"waitsplit-{n}-{ins.name}", ins=[], outs=[]
                        )
                        nop.engine = ins.engine
                        nop.sync_info = mybir.SyncInfo(on_wait=[w], on_update=[])
                        nc.register_instruction(nop, overwrite=True)
                        newlist.append(nop)
                    ins.sync_info = mybir.SyncInfo(
                        on_wait=keep, on_update=list(si.on_update)
                    )
                newlist.append(ins)
            blk.instructions[:] = newlist
    return n


HEAD_B2 = 8  # leading B2 groups (fast pipeline head)
N_B2 = 48  # single-fp8 groups total (error budget ~1%)
N_B1 = 5  # fp8 hi|lo groups


def _routes():
    """Per-group route list ('a' | 'b1' | 'b2'), identical on host/device."""
    routes = ["a"] * N_GROUPS
    for t in range(HEAD_B2):
        routes[t] = "b2"
    rest = N_B1 + N_B2 - HEAD_B2  # B groups to spread over t >= HEAD_B2
    nrest = N_GROUPS - HEAD_B2
    bidx = [i for i in range(nrest) if ((i + 1) * rest) % nrest < rest][:rest]
    for i in bidx:
        routes[HEAD_B2 + i] = "b2"
    k = max(1, len(bidx) // max(1, N_B1))
    nb1 = 0
    for j in range(k // 2, len(bidx), k):
        if nb1 < N_B1:
            routes[HEAD_B2 + bidx[j]] = "b1"
            nb1 += 1
    return routes


ROUTES = _routes()
_XT_ROWS = {"a": 0, "b1": 4, "b2": 2}
XT_OFF = []  # per-group row offset into xt_d
_o = 0
for _t in range(N_GROUPS):
    XT_OFF.append(_o)
    _o += _XT_ROWS[ROUTES[_t]]
XT_SLOTS = _o


def build_nc(kcols, split=True):
    f32 = mybir.dt.float32
    bf16 = mybir.dt.bfloat16
    ncols = kcols * N_CHUNKS  # pooling output columns
    wincols = kcols * QUAD * GROUP  # pooling columns per window (24)

    fp8 = mybir.dt.float8e4
    nc = bass.Bass()
    x_d = nc.declare_dram_parameter("x", [CHUNK, N_CHUNKS, DCX], bf16, isOutput=False)
    # packed x^T stream: per B1 group rows [kh0hi,kh0lo,kh1hi,kh1lo], per B2
    # group rows [kh0hi, kh1hi] (operands scaled by 16)
    xt_d = nc.declare_dram_parameter("xt", [CHUNK, XT_SLOTS, NPG], fp8, isOutput=False)
    # W1 fp8: per (hi|lo, m-half) a [128, 2, 128] DoubleRow block (scaled 16)
    cb8_d = nc.declare_dram_parameter("cb8", [CHUNK, 4, 2, CHUNK], fp8, isOutput=False)
    # constants packed into one bf16 and one f32 DMA:
    # cb16 = [iota | ident | w1 | w2 | bid], cf32 = [b1]
    nb16 = kcols + CHUNK + 4 * CHUNK + 2 + N_CHUNKS
    cb_d = nc.declare_dram_parameter("cb16", [CHUNK, nb16], bf16, isOutput=False)
    cf_d = nc.declare_dram_parameter("cf32", [CHUNK, 1], f32, isOutput=False)
    out_d = nc.declare_dram_parameter("out", [CHUNK, 2, ncols], bf16, isOutput=True)
    e_d = nc.declare_dram_parameter("e", [CHUNK, N_CHUNKS], bf16, isOutput=True)

    with tile.TileContext(nc) as tc, ExitStack() as ctx:
        const = ctx.enter_context(tc.tile_pool(name="const", bufs=1))
        xwp = ctx.enter_context(tc.tile_pool(name="xw", bufs=13))
        xtp = ctx.enter_context(tc.tile_pool(name="xts", bufs=8))
        hp = ctx.enter_context(tc.tile_pool(name="h", bufs=4))
        ohp = ctx.enter_context(tc.tile_pool(name="oh", bufs=3))
        sab = ctx.enter_context(tc.tile_pool(name="sab", bufs=1))
        ps_u = ctx.enter_context(tc.tile_pool(name="ps_u", bufs=3, space="PSUM"))
        ps_xt = ctx.enter_context(tc.tile_pool(name="ps_xt", bufs=1, space="PSUM"))
        ps_cw = ctx.enter_context(tc.tile_pool(name="ps_cw", bufs=1, space="PSUM"))

        # Resident constants (packed views; DMAs issued in first-use order)
        cb = const.tile([CHUNK, nb16], bf16, tag="cb16", name="cb16")
        cb8 = const.tile([CHUNK, 4, 2, CHUNK], fp8, tag="cb8", name="cb8")
        cf = const.tile([CHUNK, 1], f32, tag="cf32", name="cf32")
        iota_t = cb[:, 0:kcols]
        ident_t = cb[:, kcols : kcols + CHUNK]
        w1_base = kcols + CHUNK

        def w1_blk(s):
            return cb[:, w1_base + s * CHUNK : w1_base + (s + 1) * CHUNK]

        w2_t = cb[:, w1_base + 4 * CHUNK : w1_base + 4 * CHUNK + 2]
        bid_t = cb[:, w1_base + 4 * CHUNK + 2 : w1_base + 4 * CHUNK + 2 + N_CHUNKS]
        b1_t = cf[:, 0:1]

        # PSUM budget (8 banks): triple-buffered u tiles (6 banks; the WAR
        # distance of the tile-granular dependency tracking is then 3 tanh
        # ops), one transpose-staging bank (route-A groups are never
        # adjacent, so its reuse WAR is always two groups back), and one
        # bank holding the score window (rows 0-1) + pool window (rows 2-3).
        cw = ps_cw.tile([CHUNK, 4, wincols], f32, tag="cw", name="cw")

        # Persistent e slab and output slab (SBUF)
        e_slab = sab.tile([CHUNK, N_CHUNKS], bf16, tag="e_slab", name="e_slab")
        psab = sab.tile([CHUNK, 2, ncols], bf16, tag="psab", name="psab")

        xw_tiles = {}  # pair index -> tile
        hT_tiles = {}  # group -> tile
        xts_tiles = {}  # group -> SBUF x^T tile
        u_tiles = {}

        def get_u(t):
            u_ps = u_tiles.get(t)
            if u_ps is None:
                u_ps = ps_u.tile([CHUNK, 2, NPG], f32, tag="u", name="u")
                u_tiles[t] = u_ps
            return u_ps

        def pool_burst(q, groups):
            """onehot + pooling matmuls + window flush for a quad."""
            nchk = GROUP * len(groups)
            c0 = groups[0] * GROUP
            pw = cw[:, 2:4, :]
            # masked-e "onehot" for the whole quad in two DVE ops:
            # oh[p, i, j] = (bid[p, c0+i] == j) * e[p, i]
            shp = [CHUNK, nchk, kcols]
            oh = ohp.tile(shp, bf16, tag="oh", name="oh")
            nc.vector.tensor_tensor(
                oh[:],
                bid_t[:, c0 : c0 + nchk].unsqueeze(2).broadcast_to(shp),
                iota_t.unsqueeze(1).broadcast_to(shp),
                mybir.AluOpType.is_equal,
            )
            nc.vector.tensor_tensor(
                oh[:],
                oh[:],
                e_slab[:, c0 : c0 + nchk].unsqueeze(2).broadcast_to(shp),
                mybir.AluOpType.mult,
            )
            for i in range(nchk):
                cc = c0 + i
                pr, off = cc // (2 * GROUP), cc % (2 * GROUP)
                xw = xw_tiles[pr]
                col = kcols * i
                for half in range(2):
                    nc.tensor.matmul(
                        pw[:, half, col : col + kcols],
                        xw[:, off, half * CHUNK : (half + 1) * CHUNK],
                        oh[:, i, :],
                        start=True,
                        stop=True,
                        skip_group_check=True,
                    )
            # flush window to the SBUF slab; must run on DVE (GPSIMD cannot
            # read PSUM), but the window's matmuls are long done by now so
            # the wait rarely backs up the xts copies behind it
            nc.vector.tensor_copy(
                psab[:, :, kcols * c0 : kcols * (c0 + nchk)], pw[:, :, 0 : kcols * nchk]
            )

        def do_dma_x(t, ng=2):
            """x (natural layout) DMA covering groups [t, t+ng)."""
            pr = t // 2
            ng = min(ng, N_GROUPS - t)
            xw = xwp.tile([CHUNK, 2 * GROUP, DCX], bf16, tag="xw", name=f"xw{pr}")
            nc.sync.dma_start(
                xw[:, 0 : ng * GROUP, :],
                x_d[:, t * GROUP : (t + ng) * GROUP, :],
            )
            xw_tiles[pr] = xw

        def do_dma_xt(t):
            """Packed fp8 x^T DMA for a route-B group."""
            r = ROUTES[t]
            rows = _XT_ROWS[r]
            xts = xtp.tile([CHUNK, rows, NPG], fp8, tag=f"xt{rows}", name="xt8")
            nc.sync.dma_start(xts[:], xt_d[:, XT_OFF[t] : XT_OFF[t] + rows, :])
            xts_tiles[t] = xts

        def do_xts(t):
            """Transposes + PSUM->SBUF copy for route-A groups, one group
            ahead of the u matmuls that consume the result.  The staging bank
            holds two half-group tags so the next group's transposes only
            wait on the matching half's copy."""
            if ROUTES[t] != "a":
                return
            xw = xw_tiles[t // 2]
            off0 = (t % 2) * GROUP
            xk = get_u(t)[:, 0, :].bitcast(bf16)  # staging view in own u tile
            halves = []
            for k in range(2):
                for j in range(GROUP):
                    nc.tensor.transpose(
                        xk[:, k * NPG + j * CHUNK : k * NPG + (j + 1) * CHUNK],
                        xw[:, off0 + j, k * CHUNK : (k + 1) * CHUNK],
                        ident_t[:],
                    )
                xh = xtp.tile([CHUNK, NPG], bf16, tag=f"xts{k}", name=f"xts{k}")
                nc.vector.tensor_copy(xh[:], xk[:, k * NPG : (k + 1) * NPG])
                halves.append(xh)
            xts_tiles[t] = halves

        def do_group(t):
            """Layer-1 matmuls for group t into a rotating u tile (u=256*u)."""
            xts = xts_tiles.pop(t)
            u_ps = get_u(t)
            r = ROUTES[t]
            if r == "a":
                for k in range(2):
                    for m in range(2):
                        nc.tensor.matmul(
                            u_ps[:, m, :],
                            w1_blk(2 * k + m),
                            xts[k][:],
                            start=(k == 0),
                            stop=(k == 1),
                        )
            elif r == "b1":
                # hi*hi + hi*lo + lo*hi of 16x-scaled operands
                for m in range(2):
                    for i, (wi, xi) in enumerate(((0, 0), (1, 0), (0, 1))):
                        nc.tensor.matmul(
                            u_ps[:, m, :],
                            cb8[:, 2 * wi + m, :, :],
                            xts[:, xi::2, :],
                            start=(i == 0),
                            stop=(i == 2),
                            perf_mode=mybir.MatmulPerfMode.DoubleRow,
                        )
            else:  # b2: hi-only x
                for m in range(2):
                    for wi in range(2):
                        nc.tensor.matmul(
                            u_ps[:, m, :],
                            cb8[:, 2 * wi + m, :, :],
                            xts[:],
                            start=(wi == 0),
                            stop=(wi == 1),
                            perf_mode=mybir.MatmulPerfMode.DoubleRow,
                        )

        def do_tanh(t):
            """tanh for group t (u pool depth 3 gives 2-group lookahead)."""
            u_ps = u_tiles.pop(t)
            hT = hp.tile([CHUNK, 2, NPG], bf16, tag="hT", name="hT")
            nc.scalar.activation(
                hT[:],
                u_ps[:],
                mybir.ActivationFunctionType.Tanh,
                bias=b1_t[:, 0:1],
                scale=1.0 / 256.0,
            )
            hT_tiles[t] = hT

        s_tiles = {}  # exp-block -> ping-pong score-window tile
        blk_of_group = {}
        blk_c0 = {}

        def do_scores(t):
            hT = hT_tiles.pop(t)
            blk = blk_of_group[t]
            for j in range(GROUP):
                wc = t * GROUP + j - blk_c0[blk]
                for m in range(2):
                    nc.tensor.matmul(
                        cw[:, wc // wincols, wc % wincols : wc % wincols + 1],
                        hT[:, m, j * CHUNK : (j + 1) * CHUNK],
                        w2_t[:, m : m + 1],
                        start=(m == 0),
                        stop=(m == 1),
                        skip_group_check=True,
                    )

        def do_exp(blk, c0, c1):
            nn = c1 - c0
            if nn == 2 * wincols:
                src = cw[:, 0:2, :]
            elif nn <= wincols:
                src = cw[:, 0, 0:nn]
            else:
                raise AssertionError(nn)
            nc.scalar.activation(
                e_slab[:, c0:c1], src, mybir.ActivationFunctionType.Exp
            )

        # ---- schedule ----
        quads = []
        for q0 in range(0, N_GROUPS, QUAD):
            quads.append((q0 // QUAD, list(range(q0, min(q0 + QUAD, N_GROUPS)))))
        n_quads = len(quads)
        EXPB = 4  # quads per exp instruction in steady state

        burst_at = {}
        exp_at = {}
        blocks = []
        nq = n_quads
        while nq > 0:
            sz = EXPB if nq > EXPB + 2 else (2 if nq > 2 else 1)
            blocks.append(sz)
            nq -= sz
        j0 = 0
        for bi, sz in enumerate(blocks):
            blkq = list(range(j0, j0 + sz))
            j0 += sz
            last_group = quads[blkq[-1]][1][-1]
            te = last_group + 3  # one step after the block's last scores
            if last_group == N_GROUPS - 1:
                te = last_group + 2  # matches the final group's lag-1 scores
            c0 = quads[blkq[0]][1][0] * GROUP
            exp_at[te] = (bi, c0, (last_group + 1) * GROUP)
            blk_c0[bi] = c0
            for q in blkq:
                for g in quads[q][1]:
                    blk_of_group[g] = bi
            # bursts only have to beat the NEXT block's exp (shared window
            # bank), so in steady state they run 4 steps later, giving their
            # pooling x tiles more DMA slack; near the drain they stay tight
            lag0 = 5 if bi < len(blocks) - 3 else 1
            for i, q in enumerate(blkq):
                burst_at.setdefault(te + lag0 + i, []).append(q)
        t_end = max(burst_at) + 1
        sc_at = {}
        for g in range(N_GROUPS):
            if g == N_GROUPS - 1:
                lag = 1  # drain: PE is idle, shorten the last exp's chain
            elif g == blk_c0[blk_of_group[g]] // GROUP and g > 0:
                lag = 3
            else:
                lag = 2
            sc_at.setdefault(g + lag, []).append(g)

        flushed = [0]  # pooling columns already flushed / dmaed out

        def out_flush(upto_col):
            # issued on the otherwise-idle GPSIMD engine so the wait for
            # pooling columns never blocks SP's input-DMA stream
            a = flushed[0]
            if upto_col > a:
                nc.gpsimd.dma_start(out_d[:, :, a:upto_col], psab[:, :, a:upto_col])
                flushed[0] = upto_col

        # DMA job list ordered by first-use step
        burst_step = {}
        for ts, qs in burst_at.items():
            for q in qs:
                for g in quads[q][1]:
                    burst_step[g] = ts
        jobs = []
        for p in range(-(-N_GROUPS // 2)):
            if p == HEAD_B2 // 2:
                continue  # issued in the head sequence
            g0, g1 = 2 * p, min(2 * p + 1, N_GROUPS - 1)
            # pool bursts gate the next block's exp through the shared
            # window bank, so pooling x tiles need a generous DMA lead
            pool_need = min(burst_step[g0], burst_step[g1]) - 6
            if p >= HEAD_B2 // 2 and (ROUTES[g0] == "a" or ROUTES[g1] == "a"):
                need = min(2 * p - 3, pool_need)
            else:
                need = pool_need
            jobs.append((need, 0, ("x", 2 * p)))
        for t in range(HEAD_B2, N_GROUPS):
            if ROUTES[t] != "a":
                jobs.append((t - 3, 1, ("xt", t)))
        jobs.sort(key=lambda j: (j[0], j[1]))
        jp = [0]
        PF = 5

        def issue_jobs(t):
            while jp[0] < len(jobs) and jobs[jp[0]][0] <= t + PF:
                kind, g = jobs[jp[0]][2]
                if kind == "x":
                    do_dma_x(g)
                else:
                    do_dma_xt(g)
                jp[0] += 1

        # Head: constants + the B2 head groups' tiny xt tiles first.
        nc.sync.dma_start(cb8[:], cb8_d[:])
        do_dma_xt(0)
        do_dma_xt(1)
        nc.sync.dma_start(cf[:], cf_d[:])
        do_dma_xt(2)
        do_dma_xt(3)
        nc.sync.dma_start(cb[:], cb_d[:])
        do_dma_x(HEAD_B2)  # first route-A pair's x, ahead of its transposes
        for td in range(4, HEAD_B2):
            do_dma_xt(td)

        for t in range(0, t_end + 1):
            issue_jobs(t)
            if t in exp_at:
                do_exp(*exp_at[t])
            if t == 0:
                do_xts(0)
            # claim u tiles in strict group order so the 3-buffer rotation's
            # WAR distance is always exactly 3 tanh ops
            if t < N_GROUPS:
                get_u(t)
            # transposes for t+1 first: their DVE copy then overlaps u(t), so
            # u(t+1) is ready exactly one tanh later
            if t + 1 < N_GROUPS:
                do_xts(t + 1)
            if t < N_GROUPS:
                do_group(t)
            if t < N_GROUPS:
                do_tanh(t)
            # lag 2 so the hT wait never delays the next group's u matmuls
            # queued behind it in PE's in-order stream; the first group of
            # each exp block lags one more step so its window WAR on the
            # previous block's exp is already resolved
            for g in sc_at.get(t, ()):
                do_scores(g)
            for q in burst_at.get(t, ()):
                pool_burst(*quads[q])
                if q % 8 == 7 and n_quads - q > 4:
                    # (skipped near the drain: a GPSIMD flush there would
                    # delay the final bursts' oh builds on Pool's in-order
                    # queue)
                    out_flush(min(ncols, kcols * QUAD * GROUP * (q + 1)))
                elif q == n_quads - 2:
                    # penultimate window via SP so only the last quad's
                    # columns remain for the final flush
                    a = flushed[0]
                    b = kcols * QUAD * GROUP * (q + 1)
                    nc.sync.dma_start(out_d[:, :, a:b], psab[:, :, a:b])
                    flushed[0] = b
            if t == N_GROUPS + 1:
                # bulk of the e slab; only the final block's columns remain
                nc.sync.dma_start(e_d[:, 0:480], e_slab[:, 0:480])
        nc.sync.dma_start(e_d[:, 480:N_CHUNKS], e_slab[:, 480:N_CHUNKS])
        a = flushed[0]
        nc.sync.dma_start(out_d[:, :, a:ncols], psab[:, :, a:ncols])

    if split:
        _split_waits(nc)
    return nc


def prepare_inputs(x, batch, W1, b1, W2, b2):
    """Host-side sharding and layout preparation."""
    x = np.asarray(x, dtype=np.float32)
    batch = np.asarray(batch).astype(np.int64)
    W1 = np.asarray(W1, dtype=np.float32)
    b1 = np.asarray(b1, dtype=np.float32).reshape(D)
    W2 = np.asarray(W2, dtype=np.float32).reshape(D)

    # Pair d_out dims by sorted b1 so one per-partition bias serves both
    # tanh output halves (pairing error ~1e-4, far below bf16 noise).
    perm = np.argsort(b1, kind="stable")
    colmap = np.empty(D, np.int64)
    for m in range(2):
        colmap[m * CHUNK : (m + 1) * CHUNK] = perm[m::2]
    W1P = W1[:, colmap].astype(BF16)
    b1s = b1[perm]
    b1bar = ((b1s[0::2] + b1s[1::2]) / 2).astype(np.float32).reshape(CHUNK, 1)
    # route-A W1 blocks host-scaled by 256 so tanh's 1/256 scale is uniform
    w1t = np.empty((CHUNK, 4, CHUNK), BF16)
    ws256 = (W1P.astype(np.float32) * 256.0).astype(BF16)
    for k in range(2):
        for m in range(2):
            w1t[:, 2 * k + m, :] = ws256[
                k * CHUNK : (k + 1) * CHUNK, m * CHUNK : (m + 1) * CHUNK
            ]
    # fp8 hi|lo split of 16*W1P for the DoubleRow path
    FP8 = ml_dtypes.float8_e4m3fn
    ws = W1P.astype(np.float32) * 16.0
    w8 = [ws.astype(FP8)]
    w8.append((ws - w8[0].astype(np.float32)).astype(FP8))
    cb8 = np.empty((CHUNK, 4, 2, CHUNK), FP8)
    for wi in range(2):
        for m in range(2):
            for kh in range(2):
                cb8[:, 2 * wi + m, kh, :] = w8[wi][
                    kh * CHUNK : (kh + 1) * CHUNK, m * CHUNK : (m + 1) * CHUNK
                ]
    w2t = np.ascontiguousarray(
        W2[colmap].astype(BF16).reshape(2, CHUNK).T
    )  # w2t[p, m] = W2[colmap[m*128+p]]

    ident = np.eye(CHUNK, dtype=BF16)

    in_maps = []
    gmaps = []
    kcols_all = 1
    cores = []
    for c in range(N_CORES):
        r0 = c * NPC
        r1 = min(N_NODES, r0 + NPC)
        n = r1 - r0
        arr = np.zeros((NPAD, DCX), dtype=BF16)
        arr[:n, :D] = x[r0:r1].astype(BF16)
        x_nat = np.ascontiguousarray(
            arr.reshape(N_CHUNKS, CHUNK, DCX).transpose(1, 0, 2)
        )
        # fp8 hi|lo split of 16*x (from the same bf16 values the pool uses)
        xsc = arr.astype(np.float32) * 16.0
        xhi = xsc.astype(FP8)
        xlo = (xsc - xhi.astype(np.float32)).astype(FP8)
        # per-group packed x^T rows: [kc(128), rows, n] with rows
        # B1: [kh0hi, kh0lo, kh1hi, kh1lo], B2: [kh0hi, kh1hi]
        xt_all = np.empty((CHUNK, XT_SLOTS, NPG), FP8)
        hi_g = xhi.reshape(N_GROUPS, NPG, 2, CHUNK)  # [g, n, kh, kc]
        lo_g = xlo.reshape(N_GROUPS, NPG, 2, CHUNK)
        for t in range(N_GROUPS):
            r = ROUTES[t]
            if r == "a":
                continue
            o = XT_OFF[t]
            if r == "b1":
                for kh in range(2):
                    xt_all[:, o + 2 * kh, :] = hi_g[t, :, kh, :].T
                    xt_all[:, o + 2 * kh + 1, :] = lo_g[t, :, kh, :].T
            else:
                for kh in range(2):
                    xt_all[:, o + kh, :] = hi_g[t, :, kh, :].T

        b = batch[r0:r1]
        b_pad = np.full(NPAD, -1, np.int64)
        b_pad[:n] = b
        gf = b_pad[::CHUNK].copy()  # first graph id per chunk (-1 if pad)
        cidx = np.arange(NPAD) // CHUNK
        gf_c = np.where(gf >= 0, gf, 0)
        bid = np.where(b_pad >= 0, b_pad - gf_c[cidx], -1).astype(np.int64)
        kc = int(bid.max()) + 1
        kcols_all = max(kcols_all, kc)
        cores.append(
            (x_nat, xt_all, bid.astype(np.float32).reshape(N_CHUNKS, CHUNK).T, gf)
        )

    kcols = max(2, kcols_all)
    iota = np.broadcast_to(np.arange(kcols, dtype=BF16), (CHUNK, kcols))
    cb_common = np.concatenate(
        [iota, ident, w1t.reshape(CHUNK, 4 * CHUNK), w2t], axis=1
    ).astype(BF16)
    for c in range(N_CORES):
        x_nat, xt_all, bid2d, gf = cores[c]
        cb16 = np.concatenate([cb_common, bid2d.astype(BF16)], axis=1)
        in_maps.append(
            {
                "x": x_nat,
                "xt": xt_all,
                "cb16": np.ascontiguousarray(cb16),
                "cb8": cb8,
                "cf32": np.ascontiguousarray(b1bar),
            }
        )
        # host mapping: column kcols*c + j -> graph gf[c] + j
        gmap = np.full((N_CHUNKS, kcols), -1, np.int64)
        for j in range(kcols):
            gj = gf + j
            gmap[:, j] = np.where((gf >= 0) & (gj < G), gj, -1)
        gmaps.append(gmap)
    return in_maps, gmaps, kcols


def postprocess(results, gmaps, batch, kcols):
    batch = np.asarray(batch).astype(np.int64)
    pool = np.zeros((G, D), np.float64)
    den = np.zeros(G, np.float64)
    for c in range(N_CORES):
        res = np.asarray(results[c]["out"], dtype=np.float64)  # [128, 2, ncols]
        gm = gmaps[c].ravel()
        valid = gm >= 0
        idx = gm[valid]
        np.add.at(pool[:, :CHUNK], idx, res[:, 0, valid].T)
        np.add.at(pool[:, CHUNK:], idx, res[:, 1, valid].T)
        # denominators from the per-node e values (same bf16 values the
        # device pooled with)
        e_arr = np.asarray(results[c]["e"], dtype=np.float64)  # [128, n_chunks]
        e_node = e_arr.T.ravel()  # node order within this core
        r0 = c * NPC
        r1 = min(N_NODES, r0 + NPC)
        np.add.at(den, batch[r0:r1], e_node[: r1 - r0])
    out = np.where(den[:, None] > 0, pool / np.maximum(den, 1e-300)[:, None], 0.0)
    return out.astype(np.float32)


def kernel(x, batch, num_graphs, W1, b1, W2, b2):
    assert int(num_graphs) == G
    in_maps, gmaps, kcols = prepare_inputs(x, batch, W1, b1, W2, b2)
    nc = build_nc(kcols)
    res = run_bass_kernel_spmd(nc, in_maps, list(range(N_CORES)))
    return postprocess(res.results, gmaps, batch, kcols)


# revision 52
# speedup vs baseline: 1.0599x; 1.0046x over previous
"""AttentionReadout kernel for 8 trn2 NeuronCores (v3).

Problem: gate-MLP attention readout over 500k nodes, D=256, G=1024 graphs.
    h = tanh(x @ W1 + b1); s = h @ W2 + b2
    attn = segment_softmax(s, batch); out[g] = sum_{n in g} attn[n] * x[n]
(b2 cancels inside the per-graph softmax, so it is dropped.)

v3 design notes (bf16 data / f32 PSUM accumulation):
  - Node-sharding: 8 equal slabs of 62500 nodes padded to 62976 (123 groups
    of 512) -> one SPMD program for all cores.  Softmax normalization and
    graph gathering happen on the host from unnormalized partial sums and
    the per-node e values, so graphs split across cores/chunks are exact.
  - Three per-group routes balance PE against the DMA engines:
      A : x^T by PE-transposing the bf16 x tiles (+DVE copy to SBUF);
          layer 1 as 4 bf16 matmuls (W1 host-scaled by 256 so one tanh
          scale serves all routes).
      B1: DMA a host-pretransposed fp8 hi|lo pair (same bytes as bf16);
          layer 1 as 3 fp8 DoubleRow matmuls of 16x-scaled operands --
          bf16-accurate at half the PE cycles of route A.
      B2: DMA only the fp8 hi part (half the bytes of B1); layer 1 as 2
          DoubleRow matmuls.  The ~2.6% per-element x quantization noise
          only perturbs gate scores (softmax weights), contributing ~1%
          relative output error; pooling still uses exact bf16 x.
  - The Act engine is the critical resource (123 per-group tanh ops run
    back to back); u tiles are triple-buffered in PSUM (6 banks) so, with
    the tile-granular dependency tracking, each group's layer-1 matmuls
    only wait on the tanh three groups back.  Route-A transposes stage the
    k0 half in the group's own u tile (bitcast bf16 view, overwritten by
    the u matmuls right after the copy drains it) and the k1 half in a
    dedicated staging bank; the two PSUM->SBUF half copies pipeline with
    the transposes so the transpose->copy->matmul chain fits inside the
    three-tanh budget.  d_out is permuted by argsort(b1) and paired so
    both tanh output halves share a single per-partition bias.
  - Scores: per-chunk 1-column matmuls (start/stop per column, no memset)
    into a 48-column PSUM window sharing the last bank with the pooling
    window; exp runs per ~4 quads into a persistent e slab that is also an
    output (the host builds softmax denominators from it).  Scores lag
    their tanh by 2 steps (3 at block starts) so neither the hT wait nor
    the window WAR on the previous exp ever stalls PE's in-order stream
    ahead of the next group's u matmuls.
  - Pooling: per-chunk columns.  Each 128-node chunk spans <=kcols (=2)
    graphs; a [128, kcols] masked-e "onehot" (built 12 chunks at a time in
    two broadcast DVE ops) is the *moving* operand against the x chunk as
    *stationary*, so pooling costs ~kcols PE cycles per chunk.  Partials
    land in the shared PSUM window (start=stop=True, each column written
    once), are flushed to an SBUF slab, and stream to DRAM via the
    otherwise-idle GPSIMD DGE.  Bursts run ~5 steps behind their exp so
    their x tiles and the window-bank WAR with the next exp never gate the
    tanh stream; the drain keeps them tight instead.
  - DMAs are issued in first-use order (route-A x pairs ahead of their
    transposes, pooling-only x pairs much later); the head runs on B2
    groups whose tiny xt DMAs fill the pipeline fastest.
"""

import sys

sys.path.insert(0, "/opt/trn_rl_repo")

from contextlib import ExitStack

import numpy as np
import ml_dtypes

import concourse.bass as bass
import concourse.tile as tile
from concourse import mybir
from concourse.bass_utils import run_bass_kernel_spmd

N_NODES = 500_000
D = 256
G = 1024
N_CORES = 8
CHUNK = 128
GROUP = 4  # chunks per group (512 nodes)
NPG = CHUNK * GROUP
QUAD = 3  # groups per pool window/burst (12 chunks)
DCX = D  # x row (denominators come from the e output on host)
NPC = N_NODES // N_CORES  # 62500 real nodes per core
N_GROUPS = -(-NPC // NPG)  # 123
NPAD = N_GROUPS * NPG  # 62976
N_CHUNKS = NPAD // CHUNK  # 492
BF16 = ml_dtypes.bfloat16
SWIN = 192  # rolling score-window columns (8 sub-rows of 24)


def _split_waits(nc, max_waits=1):
    """Hoist extra semaphore waits onto preceding same-engine NOPs.

    The walrus build in this container rejects instructions carrying more
    than one embedded sync wait; engines execute their stream in order, so a
    wait on a preceding NOP is equivalent.
    """
    n = 0
    for fn in nc.m.functions:
        for blk in fn.blocks:
            newlist = []
            for ins in blk.instructions:
                si = ins.sync_info
                if si is not None and len(si.on_wait) > max_waits:
                    waits = list(si.on_wait)
                    keep, extra = waits[:max_waits], waits[max_waits:]
                    for w in extra:
                        n += 1
                        nop = mybir.InstNoOp(
                            name=f"waitsplit-{n}-{ins.name}", ins=[], outs=[]
                        )
                        nop.engine = ins.engine
                        nop.sync_info = mybir.SyncInfo(on_wait=[w], on_update=[])
                        nc.register_instruction(nop, overwrite=True)
                        newlist.append(nop)
                    ins.sync_info = mybir.SyncInfo(
                        on_wait=keep, on_update=list(si.on_update)
                    )
                newlist.append(ins)
            blk.instructions[:] = newlist
    return n


HEAD_B2 = 8  # leading B2 groups (fast pipeline head)
N_B2 = 48  # single-fp8 groups total (error budget ~1%)
N_B1 = 5  # fp8 hi|lo groups


def _routes():
    """Per-group route list ('a' | 'b1' | 'b2'), identical on host/device."""
    routes = ["a"] * N_GROUPS
    for t in range(HEAD_B2):
        routes[t] = "b2"
    rest = N_B1 + N_B2 - HEAD_B2  # B groups to spread over t >= HEAD_B2
    nrest = N_GROUPS - HEAD_B2
    bidx = [i for i in range(nrest) if ((i + 1) * rest) % nrest < rest][:rest]
    for i in bidx:
        routes[HEAD_B2 + i] = "b2"
    k = max(1, len(bidx) // max(1, N_B1))
    nb1 = 0
    for j in range(k // 2, len(bidx), k):
        if nb1 < N_B1:
            routes[HEAD_B2 + bidx[j]] = "b1"
            nb1 += 1
    return routes


ROUTES = _routes()
_XT_ROWS = {"a": 0, "b1": 4, "b2": 2}
XT_OFF = []  # per-group row offset into xt_d
_o = 0
for _t in range(N_GROUPS):
    XT_OFF.append(_o)
    _o += _XT_ROWS[ROUTES[_t]]
XT_SLOTS = _o


def build_nc(kcols, split=True):
    f32 = mybir.dt.float32
    bf16 = mybir.dt.bfloat16
    ncols = kcols * N_CHUNKS  # pooling output columns
    wincols = kcols * QUAD * GROUP  # pooling columns per window (24)

    fp8 = mybir.dt.float8e4
    nc = bass.Bass()
    x_d = nc.declare_dram_parameter("x", [CHUNK, N_CHUNKS, DCX], bf16, isOutput=False)
    # packed x^T stream: per B1 group rows [kh0hi,kh0lo,kh1hi,kh1lo], per B2
    # group rows [kh0hi, kh1hi] (operands scaled by 16)
    xt_d = nc.declare_dram_parameter("xt", [CHUNK, XT_SLOTS, NPG], fp8, isOutput=False)
    # W1 fp8: per (hi|lo, m-half) a [128, 2, 128] DoubleRow block (scaled 16)
    cb8_d = nc.declare_dram_parameter("cb8", [CHUNK, 4, 2, CHUNK], fp8, isOutput=False)
    # constants packed into one bf16 and one f32 DMA:
    # cb16 = [iota | ident | w1 | w2 | bid], cf32 = [b1]
    nb16 = kcols + CHUNK + 4 * CHUNK + 2 + N_CHUNKS
    cb_d = nc.declare_dram_parameter("cb16", [CHUNK, nb16], bf16, isOutput=False)
    cf_d = nc.declare_dram_parameter("cf32", [CHUNK, 1], f32, isOutput=False)
    out_d = nc.declare_dram_parameter("out", [CHUNK, 2, ncols], bf16, isOutput=True)
    e_d = nc.declare_dram_parameter("e", [CHUNK, N_CHUNKS], bf16, isOutput=True)

    with tile.TileContext(nc) as tc, ExitStack() as ctx:
        const = ctx.enter_context(tc.tile_pool(name="const", bufs=1))
        xwp = ctx.enter_context(tc.tile_pool(name="xw", bufs=13))
        xtp = ctx.enter_context(tc.tile_pool(name="xts", bufs=8))
        hp = ctx.enter_context(tc.tile_pool(name="h", bufs=4))
        ohp = ctx.enter_context(tc.tile_pool(name="oh", bufs=3))
        sab = ctx.enter_context(tc.tile_pool(name="sab", bufs=1))
        ps_u = ctx.enter_context(tc.tile_pool(name="ps_u", bufs=3, space="PSUM"))
        ps_xt = ctx.enter_context(tc.tile_pool(name="ps_xt", bufs=1, space="PSUM"))
        ps_cw = ctx.enter_context(tc.tile_pool(name="ps_cw", bufs=1, space="PSUM"))

        # Resident constants (packed views; DMAs issued in first-use order)
        cb = const.tile([CHUNK, nb16], bf16, tag="cb16", name="cb16")
        cb8 = const.tile([CHUNK, 4, 2, CHUNK], fp8, tag="cb8", name="cb8")
        cf = const.tile([CHUNK, 1], f32, tag="cf32", name="cf32")
        iota_t = cb[:, 0:kcols]
        ident_t = cb[:, kcols : kcols + CHUNK]
        w1_base = kcols + CHUNK

        def w1_blk(s):
            return cb[:, w1_base + s * CHUNK : w1_base + (s + 1) * CHUNK]

        w2_t = cb[:, w1_base + 4 * CHUNK : w1_base + 4 * CHUNK + 2]
        bid_t = cb[:, w1_base + 4 * CHUNK + 2 : w1_base + 4 * CHUNK + 2 + N_CHUNKS]
        b1_t = cf[:, 0:1]

        # PSUM budget (8 banks): triple-buffered u tiles (6 banks; the WAR
        # distance of the tile-granular dependency tracking is then 3 tanh
        # ops), one transpose-staging bank (route-A groups are never
        # adjacent, so its reuse WAR is always two groups back), and one
        # bank holding the score window (rows 0-1) + pool window (rows 2-3).
        cw = ps_cw.tile([CHUNK, 4, wincols], f32, tag="cw", name="cw")

        # Persistent e slab and output slab (SBUF)
        e_slab = sab.tile([CHUNK, N_CHUNKS], bf16, tag="e_slab", name="e_slab")
        psab = sab.tile([CHUNK, 2, ncols], bf16, tag="psab", name="psab")

        xw_tiles = {}  # pair index -> tile
        hT_tiles = {}  # group -> tile
        xts_tiles = {}  # group -> SBUF x^T tile
        u_tiles = {}

        def get_u(t):
            u_ps = u_tiles.get(t)
            if u_ps is None:
                u_ps = ps_u.tile([CHUNK, 2, NPG], f32, tag="u", name="u")
                u_tiles[t] = u_ps
            return u_ps

        def pool_burst(q, groups):
            """onehot + pooling matmuls + window flush for a quad."""
            nchk = GROUP * len(groups)
            c0 = groups[0] * GROUP
            pw = cw[:, 2:4, :]
            # masked-e "onehot" for the whole quad in two DVE ops:
            # oh[p, i, j] = (bid[p, c0+i] == j) * e[p, i]
            shp = [CHUNK, nchk, kcols]
            oh = ohp.tile(shp, bf16, tag="oh", name="oh")
            nc.vector.tensor_tensor(
                oh[:],
                bid_t[:, c0 : c0 + nchk].unsqueeze(2).broadcast_to(shp),
                iota_t.unsqueeze(1).broadcast_to(shp),
                mybir.AluOpType.is_equal,
            )
            nc.vector.tensor_tensor(
                oh[:],
                oh[:],
                e_slab[:, c0 : c0 + nchk].unsqueeze(2).broadcast_to(shp),
                mybir.AluOpType.mult,
            )
            for i in range(nchk):
                cc = c0 + i
                pr, off = cc // (2 * GROUP), cc % (2 * GROUP)
                xw = xw_tiles[pr]
                col = kcols * i
                for half in range(2):
                    nc.tensor.matmul(
                        pw[:, half, col : col + kcols],
                        xw[:, off, half * CHUNK : (half + 1) * CHUNK],
                        oh[:, i, :],
                        start=True,
                        stop=True,
                        skip_group_check=True,
                    )
            # flush window to the SBUF slab; must run on DVE (GPSIMD cannot
            # read PSUM), but the window's matmuls are long done by now so
            # the wait rarely backs up the xts copies behind it
            nc.vector.tensor_copy(
                psab[:, :, kcols * c0 : kcols * (c0 + nchk)], pw[:, :, 0 : kcols * nchk]
            )

        def do_dma_x(t, ng=2):
            """x (natural layout) DMA covering groups [t, t+ng)."""
            pr = t // 2
            ng = min(ng, N_GROUPS - t)
            xw = xwp.tile([CHUNK, 2 * GROUP, DCX], bf16, tag="xw", name=f"xw{pr}")
            nc.sync.dma_start(
                xw[:, 0 : ng * GROUP, :],
                x_d[:, t * GROUP : (t + ng) * GROUP, :],
            )
            xw_tiles[pr] = xw

        def do_dma_xt(t):
            """Packed fp8 x^T DMA for a route-B group."""
            r = ROUTES[t]
            rows = _XT_ROWS[r]
            xts = xtp.tile([CHUNK, rows, NPG], fp8, tag=f"xt{rows}", name="xt8")
            nc.sync.dma_start(xts[:], xt_d[:, XT_OFF[t] : XT_OFF[t] + rows, :])
            xts_tiles[t] = xts

        def do_xts(t):
            """Transposes + PSUM->SBUF copy for route-A groups, one group
            ahead of the u matmuls that consume the result.  The staging bank
            holds two half-group tags so the next group's transposes only
            wait on the matching half's copy."""
            if ROUTES[t] != "a":
                return
            xw = xw_tiles[t // 2]
            off0 = (t % 2) * GROUP
            xk = get_u(t)[:, 0, :].bitcast(bf16)  # staging view in own u tile
            halves = []
            for k in range(2):
                for j in range(GROUP):
                    nc.tensor.transpose(
                        xk[:, k * NPG + j * CHUNK : k * NPG + (j + 1) * CHUNK],
                        xw[:, off0 + j, k * CHUNK : (k + 1) * CHUNK],
                        ident_t[:],
                    )
                xh = xtp.tile([CHUNK, NPG], bf16, tag=f"xts{k}", name=f"xts{k}")
                nc.vector.tensor_copy(xh[:], xk[:, k * NPG : (k + 1) * NPG])
                halves.append(xh)
            xts_tiles[t] = halves

        def do_group(t):
            """Layer-1 matmuls for group t into a rotating u tile (u=256*u)."""
            xts = xts_tiles.pop(t)
            u_ps = get_u(t)
            r = ROUTES[t]
            if r == "a":
                for k in range(2):
                    for m in range(2):
                        nc.tensor.matmul(
                            u_ps[:, m, :],
                            w1_blk(2 * k + m),
                            xts[k][:],
                            start=(k == 0),
                            stop=(k == 1),
                        )
            elif r == "b1":
                # hi*hi + hi*lo + lo*hi of 16x-scaled operands
                for m in range(2):
                    for i, (wi, xi) in enumerate(((0, 0), (1, 0), (0, 1))):
                        nc.tensor.matmul(
                            u_ps[:, m, :],
                            cb8[:, 2 * wi + m, :, :],
                            xts[:, xi::2, :],
                            start=(i == 0),
                            stop=(i == 2),
                            perf_mode=mybir.MatmulPerfMode.DoubleRow,
                        )
            else:  # b2: hi-only x
                # the final group holds only 36 real nodes (in its first
                # chunk); computing just that chunk shortens the drain
                nn = CHUNK if t == N_GROUPS - 1 else NPG
                for m in range(2):
                    for wi in range(2):
                        nc.tensor.matmul(
                            u_ps[:, m, 0:nn],
                            cb8[:, 2 * wi + m, :, :],
                            xts[:, :, 0:nn],
                            start=(wi == 0),
                            stop=(wi == 1),
                            perf_mode=mybir.MatmulPerfMode.DoubleRow,
                        )

        def do_tanh(t):
            """tanh for group t (u pool depth 3 gives 2-group lookahead)."""
            u_ps = u_tiles.pop(t)
            nn = CHUNK if t == N_GROUPS - 1 else NPG
            hT = hp.tile([CHUNK, 2, NPG], bf16, tag="hT", name="hT")
            nc.scalar.activation(
                hT[:, :, 0:nn],
                u_ps[:, :, 0:nn],
                mybir.ActivationFunctionType.Tanh,
                bias=b1_t[:, 0:1],
                scale=1.0 / 256.0,
            )
            hT_tiles[t] = hT

        s_tiles = {}  # exp-block -> ping-pong score-window tile
        blk_of_group = {}
        blk_c0 = {}

        def do_scores(t):
            hT = hT_tiles.pop(t)
            blk = blk_of_group[t]
            njc = 1 if t == N_GROUPS - 1 else GROUP
            for j in range(njc):
                wc = t * GROUP + j - blk_c0[blk]
                for m in range(2):
                    nc.tensor.matmul(
                        cw[:, wc // wincols, wc % wincols : wc % wincols + 1],
                        hT[:, m, j * CHUNK : (j + 1) * CHUNK],
                        w2_t[:, m : m + 1],
                        start=(m == 0),
                        stop=(m == 1),
                        skip_group_check=True,
                    )

        def do_exp(blk, c0, c1):
            nn = c1 - c0
            if nn == 2 * wincols:
                src = cw[:, 0:2, :]
            elif nn <= wincols:
                src = cw[:, 0, 0:nn]
            else:
                raise AssertionError(nn)
            nc.scalar.activation(
                e_slab[:, c0:c1], src, mybir.ActivationFunctionType.Exp
            )

        # ---- schedule ----
        quads = []
        for q0 in range(0, N_GROUPS, QUAD):
            quads.append((q0 // QUAD, list(range(q0, min(q0 + QUAD, N_GROUPS)))))
        n_quads = len(quads)
        EXPB = 4  # quads per exp instruction in steady state

        burst_at = {}
        exp_at = {}
        blocks = []
        nq = n_quads
        while nq > 0:
            sz = EXPB if nq > EXPB + 2 else (2 if nq > 2 else 1)
            blocks.append(sz)
            nq -= sz
        j0 = 0
        for bi, sz in enumerate(blocks):
            blkq = list(range(j0, j0 + sz))
            j0 += sz
            last_group = quads[blkq[-1]][1][-1]
            te = last_group + 3  # one step after the block's last scores
            if last_group == N_GROUPS - 1:
                te = last_group + 2  # matches the final group's lag-1 scores
            c0 = quads[blkq[0]][1][0] * GROUP
            exp_at[te] = (bi, c0, (last_group + 1) * GROUP)
            blk_c0[bi] = c0
            for q in blkq:
                for g in quads[q][1]:
                    blk_of_group[g] = bi
            # bursts only have to beat the NEXT block's exp (shared window
            # bank), so in steady state they run 4 steps later, giving their
            # pooling x tiles more DMA slack; near the drain they stay tight
            lag0 = 5 if bi < len(blocks) - 3 else 1
            for i, q in enumerate(blkq):
                burst_at.setdefault(te + lag0 + i, []).append(q)
        t_end = max(burst_at) + 1
        sc_at = {}
        for g in range(N_GROUPS):
            if g == N_GROUPS - 1:
                lag = 1  # drain: PE is idle, shorten the last exp's chain
            elif g == blk_c0[blk_of_group[g]] // GROUP and g > 0:
                lag = 3
            else:
                lag = 2
            sc_at.setdefault(g + lag, []).append(g)

        flushed = [0]  # pooling columns already flushed / dmaed out

        def out_flush(upto_col):
            # issued on the otherwise-idle GPSIMD engine so the wait for
            # pooling columns never blocks SP's input-DMA stream
            a = flushed[0]
            if upto_col > a:
                nc.gpsimd.dma_start(out_d[:, :, a:upto_col], psab[:, :, a:upto_col])
                flushed[0] = upto_col

        # DMA job list ordered by first-use step
        burst_step = {}
        for ts, qs in burst_at.items():
            for q in qs:
                for g in quads[q][1]:
                    burst_step[g] = ts
        jobs = []
        for p in range(-(-N_GROUPS // 2)):
            if p == HEAD_B2 // 2:
                continue  # issued in the head sequence
            g0, g1 = 2 * p, min(2 * p + 1, N_GROUPS - 1)
            # pool bursts gate the next block's exp through the shared
            # window bank, so pooling x tiles need a generous DMA lead
            pool_need = min(burst_step[g0], burst_step[g1]) - 6
            if p >= HEAD_B2 // 2 and (ROUTES[g0] == "a" or ROUTES[g1] == "a"):
                need = min(2 * p - 3, pool_need)
            else:
                need = pool_need
            jobs.append((need, 0, ("x", 2 * p)))
        for t in range(HEAD_B2, N_GROUPS):
            if ROUTES[t] != "a":
                jobs.append((t - 3, 1, ("xt", t)))
        jobs.sort(key=lambda j: (j[0], j[1]))
        jp = [0]
        PF = 5

        def issue_jobs(t):
            while jp[0] < len(jobs) and jobs[jp[0]][0] <= t + PF:
                kind, g = jobs[jp[0]][2]
                if kind == "x":
                    do_dma_x(g)
                else:
                    do_dma_xt(g)
                jp[0] += 1

        # Head: constants + the B2 head groups' tiny xt tiles first.
        nc.sync.dma_start(cb8[:], cb8_d[:])
        do_dma_xt(0)
        do_dma_xt(1)
        nc.sync.dma_start(cf[:], cf_d[:])
        do_dma_xt(2)
        do_dma_xt(3)
        nc.sync.dma_start(cb[:], cb_d[:])
        do_dma_x(HEAD_B2)  # first route-A pair's x, ahead of its transposes
        for td in range(4, HEAD_B2):
            do_dma_xt(td)

        for t in range(0, t_end + 1):
            issue_jobs(t)
            if t in exp_at:
                do_exp(*exp_at[t])
            if t == 0:
                do_xts(0)
            # claim u tiles in strict group order so the 3-buffer rotation's
            # WAR distance is always exactly 3 tanh ops
            if t < N_GROUPS:
                get_u(t)
            # transposes for t+1 first: their DVE copy then overlaps u(t), so
            # u(t+1) is ready exactly one tanh later
            if t + 1 < N_GROUPS:
                do_xts(t + 1)
            if t < N_GROUPS:
                do_group(t)
            if t < N_GROUPS:
                do_tanh(t)
            # lag 2 so the hT wait never delays the next group's u matmuls
            # queued behind it in PE's in-order stream; the first group of
            # each exp block lags one more step so its window WAR on the
            # previous block's exp is already resolved
            for g in sc_at.get(t, ()):
                do_scores(g)
            for q in burst_at.get(t, ()):
                pool_burst(*quads[q])
                if q % 8 == 7 and n_quads - q > 4:
                    # (skipped near the drain: a GPSIMD flush there would
                    # delay the final bursts' oh builds on Pool's in-order
                    # queue)
                    out_flush(min(ncols, kcols * QUAD * GROUP * (q + 1)))
                elif q == n_quads - 2:
                    # penultimate window via SP so only the last quad's
                    # columns remain for the final flush
                    a = flushed[0]
                    b = kcols * QUAD * GROUP * (q + 1)
                    nc.sync.dma_start(out_d[:, :, a:b], psab[:, :, a:b])
                    flushed[0] = b
            if t == N_GROUPS + 1:
                # bulk of the e slab; only the final block's columns remain
                nc.sync.dma_start(e_d[:, 0:480], e_slab[:, 0:480])
        nc.sync.dma_start(e_d[:, 480:N_CHUNKS], e_slab[:, 480:N_CHUNKS])
        a = flushed[0]
        nc.sync.dma_start(out_d[:, :, a:ncols], psab[:, :, a:ncols])

    if split:
        _split_waits(nc)
    return nc


def prepare_inputs(x, batch, W1, b1, W2, b2):
    """Host-side sharding and layout preparation."""
    x = np.asarray(x, dtype=np.float32)
    batch = np.asarray(batch).astype(np.int64)
    W1 = np.asarray(W1, dtype=np.float32)
    b1 = np.asarray(b1, dtype=np.float32).reshape(D)
    W2 = np.asarray(W2, dtype=np.float32).reshape(D)

    # Pair d_out dims by sorted b1 so one per-partition bias serves both
    # tanh output halves (pairing error ~1e-4, far below bf16 noise).
    perm = np.argsort(b1, kind="stable")
    colmap = np.empty(D, np.int64)
    for m in range(2):
        colmap[m * CHUNK : (m + 1) * CHUNK] = perm[m::2]
    W1P = W1[:, colmap].astype(BF16)
    b1s = b1[perm]
    b1bar = ((b1s[0::2] + b1s[1::2]) / 2).astype(np.float32).reshape(CHUNK, 1)
    # route-A W1 blocks host-scaled by 256 so tanh's 1/256 scale is uniform
    w1t = np.empty((CHUNK, 4, CHUNK), BF16)
    ws256 = (W1P.astype(np.float32) * 256.0).astype(BF16)
    for k in range(2):
        for m in range(2):
            w1t[:, 2 * k + m, :] = ws256[
                k * CHUNK : (k + 1) * CHUNK, m * CHUNK : (m + 1) * CHUNK
            ]
    # fp8 hi|lo split of 16*W1P for the DoubleRow path
    FP8 = ml_dtypes.float8_e4m3fn
    ws = W1P.astype(np.float32) * 16.0
    w8 = [ws.astype(FP8)]
    w8.append((ws - w8[0].astype(np.float32)).astype(FP8))
    cb8 = np.empty((CHUNK, 4, 2, CHUNK), FP8)
    for wi in range(2):
        for m in range(2):
            for kh in range(2):
                cb8[:, 2 * wi + m, kh, :] = w8[wi][
                    kh * CHUNK : (kh + 1) * CHUNK, m * CHUNK : (m + 1) * CHUNK
                ]
    w2t = np.ascontiguousarray(
        W2[colmap].astype(BF16).reshape(2, CHUNK).T
    )  # w2t[p, m] = W2[colmap[m*128+p]]

    ident = np.eye(CHUNK, dtype=BF16)

    in_maps = []
    gmaps = []
    kcols_all = 1
    cores = []
    for c in range(N_CORES):
        r0 = c * NPC
        r1 = min(N_NODES, r0 + NPC)
        n = r1 - r0
        arr = np.zeros((NPAD, DCX), dtype=BF16)
        arr[:n, :D] = x[r0:r1].astype(BF16)
        x_nat = np.ascontiguousarray(
            arr.reshape(N_CHUNKS, CHUNK, DCX).transpose(1, 0, 2)
        )
        # fp8 hi|lo split of 16*x (from the same bf16 values the pool uses)
        xsc = arr.astype(np.float32) * 16.0
        xhi = xsc.astype(FP8)
        xlo = (xsc - xhi.astype(np.float32)).astype(FP8)
        # per-group packed x^T rows: [kc(128), rows, n] with rows
        # B1: [kh0hi, kh0lo, kh1hi, kh1lo], B2: [kh0hi, kh1hi]
        xt_all = np.empty((CHUNK, XT_SLOTS, NPG), FP8)
        hi_g = xhi.reshape(N_GROUPS, NPG, 2, CHUNK)  # [g, n, kh, kc]
        lo_g = xlo.reshape(N_GROUPS, NPG, 2, CHUNK)
        for t in range(N_GROUPS):
            r = ROUTES[t]
            if r == "a":
                continue
            o = XT_OFF[t]
            if r == "b1":
                for kh in range(2):
                    xt_all[:, o + 2 * kh, :] = hi_g[t, :, kh, :].T
                    xt_all[:, o + 2 * kh + 1, :] = lo_g[t, :, kh, :].T
            else:
                for kh in range(2):
                    xt_all[:, o + kh, :] = hi_g[t, :, kh, :].T

        b = batch[r0:r1]
        b_pad = np.full(NPAD, -1, np.int64)
        b_pad[:n] = b
        gf = b_pad[::CHUNK].copy()  # first graph id per chunk (-1 if pad)
        cidx = np.arange(NPAD) // CHUNK
        gf_c = np.where(gf >= 0, gf, 0)
        bid = np.where(b_pad >= 0, b_pad - gf_c[cidx], -1).astype(np.int64)
        kc = int(bid.max()) + 1
        kcols_all = max(kcols_all, kc)
        cores.append(
            (x_nat, xt_all, bid.astype(np.float32).reshape(N_CHUNKS, CHUNK).T, gf)
        )

    kcols = max(2, kcols_all)
    iota = np.broadcast_to(np.arange(kcols, dtype=BF16), (CHUNK, kcols))
    cb_common = np.concatenate(
        [iota, ident, w1t.reshape(CHUNK, 4 * CHUNK), w2t], axis=1
    ).astype(BF16)
    for c in range(N_CORES):
        x_nat, xt_all, bid2d, gf = cores[c]
        cb16 = np.concatenate([cb_common, bid2d.astype(BF16)], axis=1)
        in_maps.append(
            {
                "x": x_nat,
                "xt": xt_all,
                "cb16": np.ascontiguousarray(cb16),
                "cb8": cb8,
                "cf32": np.ascontiguousarray(b1bar),
            }
        )
        # host mapping: column kcols*c + j -> graph gf[c] + j
        gmap = np.full((N_CHUNKS, kcols), -1, np.int64)
        for j in range(kcols):
            gj = gf + j
            gmap[:, j] = np.where((gf >= 0) & (gj < G), gj, -1)
        gmaps.append(gmap)
    return in_maps, gmaps, kcols


def postprocess(results, gmaps, batch, kcols):
    batch = np.asarray(batch).astype(np.int64)
    pool = np.zeros((G, D), np.float64)
    den = np.zeros(G, np.float64)
    for c in range(N_CORES):
        res = np.asarray(results[c]["out"], dtype=np.float64)  # [128, 2, ncols]
        gm = gmaps[c].ravel()
        valid = gm >= 0
        idx = gm[valid]
        np.add.at(pool[:, :CHUNK], idx, res[:, 0, valid].T)
        np.add.at(pool[:, CHUNK:], idx, res[:, 1, valid].T)
        # denominators from the per-node e values (same bf16 values the
        # device pooled with)
        e_arr = np.asarray(results[c]["e"], dtype=np.float64)  # [128, n_chunks]
        e_node = e_arr.T.ravel()  # node order within this core
        r0 = c * NPC
        r1 = min(N_NODES, r0 + NPC)
        np.add.at(den, batch[r0:r1], e_node[: r1 - r0])
    out = np.where(den[:, None] > 0, pool / np.maximum(den, 1e-300)[:, None], 0.0)
    return out.astype(np.float32)


def kernel(x, batch, num_graphs, W1, b1, W2, b2):
    assert int(num_graphs) == G
    in_maps, gmaps, kcols = prepare_inputs(x, batch, W1, b1, W2, b2)
    nc = build_nc(kcols)
    res = run_bass_kernel_spmd(nc, in_maps, list(range(N_CORES)))
    return postprocess(res.results, gmaps, batch, kcols)
